# revision 12
# baseline (speedup 1.0000x reference)
"""Trainium2 Bass kernel for GATPolicy multitask (gnn_message_passing).

Strategy (8 NeuronCores, edge-parallel by destination range):
  - alpha decomposes per head as A_i[dst] + A_j[src] + A_e(e_feat); A_i cancels
    in the segment softmax ratio, and max-subtraction is unneeded (beta range
    is tiny), so softmax = exp(beta)/seg_sum(exp(beta)) with
    beta = A_j[src] + A_e(e).
  - Edges sorted by dst; each core owns a contiguous dst block range, so
    segment sums are core-local (S-matrix one-hot matmuls accumulated in PSUM
    per 128-node block). One AllGather ships the layer-2 gather table.
  - x_l rows (64 f32 = 256B) gathered per edge via dma_gather with int16
    local indices over <=32768-row table subranges.
"""
import os
import sys
import numpy as np

sys.path.insert(0, "/opt/trn_rl_repo")

EMB = 64
HEADS = 8
N_CONS = int(os.environ.get("GAT_NCONS", 50000))
N_VARS = int(os.environ.get("GAT_NVARS", 100000))
N_CORES = 8


def _blocks_per_core(n):
    b = (n + 8 * 128 - 1) // (8 * 128)
    return ((b + 6) // 7) * 7  # multiple of 7 so DCHUNK divides shards


CB_CORE = _blocks_per_core(N_CONS)   # 49 at full size
VB_CORE = _blocks_per_core(N_VARS)   # 98 at full size
NC_PAD = N_CORES * CB_CORE * 128   # 50176
NV_PAD = N_CORES * VB_CORE * 128   # 100352
C_SHARD = CB_CORE * 128     # 6272
V_SHARD = VB_CORE * 128     # 12544
SUB = 32768                 # gather subrange size (int16 limit)
NSUB_V = (NV_PAD + SUB - 1) // SUB   # 4
NSUB_C = (NC_PAD + SUB - 1) // SUB   # 2
DCHUNK = 896                # dense chunk (7 blocks of 128)
GTARGET = 16                # target tiles per gather call / de group
EPS = 1e-8
LAST_EXEC_NS = None
LAST_TIMES = None

# head ranges of x_j columns (pos 64+c of concat192 -> head (64+c)//24):
# h2: cols 0:8, h3: 8:32, h4: 32:56, h5: 56:64


def _head_mats(att):
    att = np.asarray(att).reshape(HEADS, 24)
    G_j = np.zeros((64, HEADS), np.float32)
    G_e = np.zeros((64, HEADS), np.float32)
    gvec = np.zeros(64, np.float32)
    for c in range(64):
        pos = 64 + c
        h, j = pos // 24, pos % 24
        G_j[c, h] = att[h, j]
        gvec[c] = att[h, j]
        pos = 128 + c
        h, j = pos // 24, pos % 24
        G_e[c, h] = att[h, j]
    return G_j, G_e, gvec


def _lrelu(x):
    return np.where(x > 0, x, 0.2 * x)


def _prep_edges(dst, src_local_all, sub_of_src, e_feat, n_blocks_core, nsub):
    """Sort edges by dst, shard per core, pad (block, subrange) groups to a
    tile structure shared across cores.  Returns (runs, T_s, T_total, cores):
      runs[b] = [(sub, n_tiles), ...]       shared program structure
      cores[c] = (idx16 per stream, de [128, T_total, 2] f32)
    """
    order = np.argsort(dst, kind="stable")
    dst_s = dst[order]
    shard = 128 * n_blocks_core
    core_of = dst_s // shard
    blk_local = (dst_s % shard) // 128
    dst_local = dst_s % 128

    counts = np.zeros((N_CORES, n_blocks_core, nsub), np.int64)
    per_core_edge = []
    for c in range(N_CORES):
        m = core_of == c
        eo = order[m]
        bl = blk_local[m]
        dl = dst_local[m]
        sl = src_local_all[c][eo]
        sb = sub_of_src[c][eo]
        ef = e_feat[eo]
        key = bl * nsub + sb
        ko = np.argsort(key, kind="stable")
        per_core_edge.append((bl[ko], dl[ko], sl[ko], ef[ko]))
        np.add.at(counts[c], (bl[ko], sb[ko]), 1)

    tiles = (counts + 127) // 128
    T_bs = tiles.max(axis=0)  # [B, S] shared across cores
    runs = []
    for b in range(n_blocks_core):
        rr = [(s, int(T_bs[b, s])) for s in range(nsub) if T_bs[b, s] > 0]
        if not rr:
            rr = [(0, 1)]  # all-dummy tile keeps block structure uniform
            T_bs[b, 0] = 1
        runs.append(rr)

    T_s = T_bs.sum(axis=0)
    T_total = int(T_s.sum())

    cores = []
    for c in range(N_CORES):
        bl, dl, sl, ef = per_core_edge[c]
        idx_streams = [np.zeros(max(int(T_s[s]), 1) * 128, np.int16)
                       for s in range(nsub)]
        dst_arr = np.full(T_total * 128, -1.0, np.float32)
        e_arr = np.zeros(T_total * 128, np.float32)
        spos = np.zeros(nsub, np.int64)
        gpos = 0
        ptr = 0
        for b in range(n_blocks_core):
            for s, nt in runs[b]:
                cnt = int(counts[c, b, s])
                seg = slice(ptr, ptr + cnt)
                assert cnt <= nt * 128
                i0 = int(spos[s]) * 128
                idx_streams[s][i0:i0 + cnt] = sl[seg]
                g0 = gpos * 128
                dst_arr[g0:g0 + cnt] = dl[seg]
                e_arr[g0:g0 + cnt] = ef[seg]
                ptr += cnt
                spos[s] += nt
                gpos += nt
        assert ptr == bl.shape[0] and gpos == T_total
        idx16 = []
        for s in range(nsub):
            a = idx_streams[s].reshape(-1, 16).T
            idx16.append(np.ascontiguousarray(np.tile(a, (8, 1))))
        de = np.stack([dst_arr, e_arr], -1).reshape(T_total, 128, 2)
        de = np.ascontiguousarray(de.transpose(1, 0, 2))
        cores.append((idx16, de))
    return runs, T_s, T_total, cores


def _build_and_run(host):
    import concourse.bass as bass
    import concourse.bacc as bacc
    import concourse.tile as tile
    from concourse import mybir
    from concourse.bass_utils import run_bass_kernel_spmd
    from concourse.masks import make_identity

    f32 = mybir.dt.float32
    i16 = mybir.dt.int16
    i32 = mybir.dt.int32
    AF = mybir.ActivationFunctionType
    OP = mybir.AluOpType
    X = mybir.AxisListType.X

    runs1, T_s1, T1, _ = host["l1"]
    runs2, T_s2, T2, _ = host["l2"]

    nc = bacc.Bacc("TRN2", target_bir_lowering=False, debug=False,
                   num_devices=N_CORES)

    def din(name, shape, dtype=f32):
        return nc.dram_tensor(name, shape, dtype, kind="ExternalInput")

    wspec = [
        ("vW1T", [15, 64]), ("vb1", [64, 1]), ("vW2T", [64, 64]),
        ("vb2", [64, 1]), ("Wl1T", [64, 64]), ("bl1", [64, 1]),
        ("cW1T", [4, 64]), ("cb1", [64, 1]), ("cW2T", [64, 64]),
        ("cb2", [64, 1]), ("Wo1T", [128, 64]), ("bo1", [64, 1]),
        ("Wo2T", [64, 64]), ("bo2", [64, 1]), ("Wl2T", [64, 64]),
        ("bl2", [64, 1]), ("Uo1T", [128, 64]), ("ub1", [64, 1]),
        ("Uo2T", [64, 64]), ("ub2", [64, 1]), ("hW1T", [64, 64]),
        ("hb1", [64, 1]), ("hW2", [64, 1]),
    ]
    wdr = {nm: din(nm, shp) for nm, shp in wspec}
    bspec = [("g1r", 64), ("g2r", 64), ("Q1r", 8), ("PmQ1r", 8),
             ("Q2r", 8), ("PmQ2r", 8)]
    bdr = {nm: din(nm, [1, n]) for nm, n in bspec}

    vfT = din("vfT", [15, NV_PAD])
    cfT = din("cfT", [4, C_SHARD])
    idx1 = [din(f"idx1_{s}", [128, max(int(T_s1[s]), 1) * 8], i16)
            for s in range(NSUB_V)]
    idx2 = [din(f"idx2_{s}", [128, max(int(T_s2[s]), 1) * 8], i16)
            for s in range(NSUB_C)]
    de1 = din("de1", [128, T1, 2])
    de2 = din("de2", [128, T2, 2])

    out_score = nc.dram_tensor("out_score", [1, V_SHARD], f32,
                               kind="ExternalOutput")

    vtable = nc.dram_tensor("vtable", [NV_PAD, 64], f32, kind="Internal")
    vshardT = nc.dram_tensor("vshardT", [64, V_SHARD], f32, kind="Internal")
    cshardT = nc.dram_tensor("cshardT", [64, C_SHARD], f32, kind="Internal")
    c2shard = nc.dram_tensor("c2shard", [C_SHARD, 64], f32, kind="Internal")
    ctable = nc.dram_tensor("ctable", [NC_PAD, 64], f32, kind="Internal",
                            addr_space="Shared")

    def rv(base_ap, pattern):
        """Re-pattern an AP keeping its (possibly symbolic) offset."""
        return bass.AP(tensor=base_ap.tensor, offset=base_ap.offset,
                       ap=pattern)

    with tile.TileContext(nc) as tc:
        with tc.tile_pool(name="const", bufs=1) as const:
            ident = const.tile([128, 128], f32)
            make_identity(nc, ident[:])
            iota_i = const.tile([128, 128], i32)
            nc.gpsimd.iota(iota_i[:], pattern=[[1, 128]], base=0,
                           channel_multiplier=0)
            iota = const.tile([128, 128], f32)
            nc.vector.tensor_copy(out=iota[:], in_=iota_i[:])

            bts = {}
            for nm, n in bspec:
                t = const.tile([128, n], f32, tag=nm)
                nc.gpsimd.dma_start(
                    out=t[:],
                    in_=bass.AP(tensor=bdr[nm], offset=0,
                                ap=[[0, 128], [1, n]]))
                bts[nm] = t
            wts = {}
            for nm, shp in wspec:
                w = const.tile(shp, f32, tag=nm)
                nc.sync.dma_start(out=w[:], in_=wdr[nm][:])
                wts[nm] = w

            # ---------------- dense embeds (feat-major) -------------------
            with (
                tc.tile_pool(name="dloads", bufs=3) as dloads,
                tc.tile_pool(name="dwork", bufs=3) as dwork,
                tc.tile_pool(name="dpsum", bufs=2, space="PSUM") as dpsum,
            ):
                for k in range(C_SHARD // DCHUNK):
                    xin = dloads.tile([4, DCHUNK], f32, tag="cin")
                    nc.sync.dma_start(out=xin[:],
                                      in_=cfT[:, k * DCHUNK:(k + 1) * DCHUNK])
                    cT = dwork.tile([64, DCHUNK], f32, tag="cT")
                    for h in range(2):
                        sl = slice(h * 448, (h + 1) * 448)
                        p1 = dpsum.tile([64, 448], f32, tag="pa")
                        nc.tensor.matmul(out=p1[:], lhsT=wts["cW1T"][:],
                                         rhs=xin[:, sl], start=True, stop=True)
                        h1 = dwork.tile([64, 448], f32, tag="h1")
                        nc.scalar.activation(out=h1[:], in_=p1[:], func=AF.Relu,
                                             bias=wts["cb1"][:, 0:1])
                        p2 = dpsum.tile([64, 448], f32, tag="pa")
                        nc.tensor.matmul(out=p2[:], lhsT=wts["cW2T"][:],
                                         rhs=h1[:], start=True, stop=True)
                        nc.scalar.activation(out=cT[:, sl], in_=p2[:],
                                             func=AF.Relu,
                                             bias=wts["cb2"][:, 0:1])
                    nc.sync.dma_start(
                        out=cshardT[:, k * DCHUNK:(k + 1) * DCHUNK], in_=cT[:])

                for k in range(NV_PAD // DCHUNK):
                    xin = dloads.tile([15, DCHUNK], f32, tag="vin")
                    nc.sync.dma_start(out=xin[:],
                                      in_=vfT[:, k * DCHUNK:(k + 1) * DCHUNK])
                    vT = dwork.tile([64, DCHUNK], f32, tag="vT")
                    xlT = dwork.tile([64, DCHUNK], f32, tag="xlT")
                    for h in range(2):
                        sl = slice(h * 448, (h + 1) * 448)
                        p1 = dpsum.tile([64, 448], f32, tag="pa")
                        nc.tensor.matmul(out=p1[:], lhsT=wts["vW1T"][:],
                                         rhs=xin[:, sl], start=True, stop=True)
                        h1 = dwork.tile([64, 448], f32, tag="h1")
                        nc.scalar.activation(out=h1[:], in_=p1[:], func=AF.Relu,
                                             bias=wts["vb1"][:, 0:1])
                        p2 = dpsum.tile([64, 448], f32, tag="pa")
                        nc.tensor.matmul(out=p2[:], lhsT=wts["vW2T"][:],
                                         rhs=h1[:], start=True, stop=True)
                        nc.scalar.activation(out=vT[:, sl], in_=p2[:],
                                             func=AF.Relu,
                                             bias=wts["vb2"][:, 0:1])
                        p3 = dpsum.tile([64, 448], f32, tag="pa")
                        nc.tensor.matmul(out=p3[:], lhsT=wts["Wl1T"][:],
                                         rhs=vT[:, sl], start=True, stop=True)
                        nc.scalar.activation(out=xlT[:, sl], in_=p3[:],
                                             func=AF.Identity,
                                             bias=wts["bl1"][:, 0:1])
                    pt = dpsum.tile([128, 448], f32, tag="pt")
                    for t in range(7):
                        nc.tensor.transpose(
                            out=pt[:, t * 64:(t + 1) * 64],
                            in_=xlT[:, t * 128:(t + 1) * 128],
                            identity=ident[0:64, 0:64])
                    xn = dwork.tile([128, 448], f32, tag="xn")
                    nc.vector.tensor_copy(out=xn[:], in_=pt[:])
                    nc.sync.dma_start(
                        out=vtable[k * DCHUNK:(k + 1) * DCHUNK, :].rearrange(
                            "(a p) f -> p a f", p=128),
                        in_=xn[:].rearrange("p (a f) -> p a f", f=64))
                    if k < V_SHARD // DCHUNK:
                        nc.sync.dma_start(
                            out=vshardT[:, k * DCHUNK:(k + 1) * DCHUNK],
                            in_=vT[:])

            # ---------------- GAT layers ----------------------------------
            SUBSTAGE = os.environ.get("GAT_SUB", "full")

            def gat_layer(runs, nsub, idx_dr, de_dr, table, tab_rows,
                          gb, Qb, PmQb, n_blocks, epilogue):
                maxL = max(nt for rr in runs for (_, nt) in rr)
                callplan = [[] for _ in range(nsub)]
                spos = [0] * nsub
                cur = [[0, 0] for _ in range(nsub)]
                run_src = {}
                for b in range(n_blocks):
                    for s, nt in runs[b]:
                        if cur[s][1] >= GTARGET:
                            callplan[s].append(tuple(cur[s]))
                            cur[s] = [spos[s], 0]
                        run_src[(b, s)] = (len(callplan[s]), cur[s][1])
                        cur[s][1] += nt
                        spos[s] += nt
                for s in range(nsub):
                    if cur[s][1] > 0:
                        callplan[s].append(tuple(cur[s]))
                maxw = max(w for cp in callplan if cp for (_, w) in cp)
                de_groups = []
                t0, w = 0, 0
                for b in range(n_blocks):
                    w += sum(nt for (_, nt) in runs[b])
                    if w >= GTARGET:
                        de_groups.append((t0, w))
                        t0, w = t0 + w, 0
                if w > 0:
                    de_groups.append((t0, w))
                maxdw = max(w for (_, w) in de_groups)

                with (
                    tc.tile_pool(name="gidx", bufs=2) as gidx,
                    tc.tile_pool(name="gbuf", bufs=2) as gbufp,
                    tc.tile_pool(name="debuf", bufs=2) as debufp,
                    tc.tile_pool(name="ework", bufs=2) as ework,
                    tc.tile_pool(name="acc", bufs=2, space="PSUM") as accp,
                    tc.tile_pool(name="epi", bufs=1, space="PSUM") as epip,
                ):
                    call_done = [0] * nsub
                    gbufs = [None] * nsub
                    de_iter = iter(de_groups)
                    de_tile = None
                    de_w = 0
                    run_pos_in_de = 0

                    for b in range(n_blocks):
                        if de_tile is None or run_pos_in_de >= de_w:
                            g0, gw = next(de_iter)
                            de_tile = debufp.tile([128, maxdw, 2], f32,
                                                  tag="de")
                            nc.sync.dma_start(out=de_tile[:, :gw, :],
                                              in_=de_dr[:, g0:g0 + gw, :])
                            de_w = gw
                            run_pos_in_de = 0
                        acc = (None if SUBSTAGE in ("g", "e")
                               else accp.tile([128, 72], f32, tag="acc"))
                        first = True
                        tb_total = sum(nt for (_, nt) in runs[b])
                        done = 0
                        for s, nt in runs[b]:
                            cid, off = run_src[(b, s)]
                            if call_done[s] <= cid:
                                t0c, wc = callplan[s][cid]
                                it = gidx.tile([128, wc * 8], i16, tag="idx")
                                nc.sync.dma_start(
                                    out=it[:],
                                    in_=idx_dr[s][:, t0c * 8:(t0c + wc) * 8])
                                gt = gbufp.tile([128, wc * 64], f32,
                                                tag=f"g{s}")
                                sub_rows = min(SUB, tab_rows - s * SUB)
                                nc.gpsimd.dma_gather(
                                    out_ap=gt[:].rearrange(
                                        "p (a f) -> p a f", f=64),
                                    in_ap=table[s * SUB:s * SUB + sub_rows, :],
                                    idxs_ap=it[:],
                                    num_idxs=wc * 128, num_idxs_reg=wc * 128,
                                    elem_size=64, single_packet=False)
                                gbufs[s] = gt
                                call_done[s] = cid + 1
                            gt = gbufs[s]
                            L = nt
                            if SUBSTAGE == "g":
                                done += L
                                continue
                            dpos = run_pos_in_de + done
                            deb = de_tile[:, dpos:dpos + L, 0:1]
                            p0 = deb.ap[0]
                            dstv = rv(deb, [p0, [2, L], [0, 128]])
                            deb1 = de_tile[:, dpos:dpos + L, 1:2]
                            ev8 = rv(deb1, [p0, [2, L], [0, 8]])
                            ev1 = rv(deb1, [p0, [2, L]])
                            xj = gt[:, off * 64:(off + L) * 64]
                            # t1 = lrelu(xj) * g
                            t1 = ework.tile([128, maxL * 64], f32, tag="t1")
                            t1v = t1[:, :L * 64]
                            nc.scalar.mul(t1v, xj, 0.2)
                            nc.vector.tensor_tensor(out=t1v, in0=t1v, in1=xj,
                                                    op=OP.max)
                            gbv = rv(gb[:], [gb[:].ap[0], [0, L], [1, 64]])
                            nc.vector.tensor_tensor(out=t1v, in0=t1v, in1=gbv,
                                                    op=OP.mult)
                            # beta = A_e + per-head reductions of t1
                            beta = ework.tile([128, maxL, 8], f32, tag="beta")
                            bv = beta[:, :L, :]
                            Qv = rv(Qb[:], [Qb[:].ap[0], [0, L], [1, 8]])
                            PmQv = rv(PmQb[:], [PmQb[:].ap[0], [0, L], [1, 8]])
                            nc.vector.tensor_tensor(out=bv, in0=Qv, in1=ev8,
                                                    op=OP.mult)
                            re = ework.tile([128, maxL], f32, tag="re")
                            nc.scalar.activation(out=re[:, :L], in_=ev1,
                                                 func=AF.Relu)
                            tm = ework.tile([128, maxL, 8], f32, tag="tm")
                            rebase = re[:, 0:L]
                            rev = rv(rebase, [rebase.ap[0], [1, L], [0, 8]])
                            nc.vector.tensor_tensor(out=tm[:, :L, :],
                                                    in0=PmQv, in1=rev,
                                                    op=OP.mult)
                            nc.vector.tensor_tensor(out=bv, in0=bv,
                                                    in1=tm[:, :L, :],
                                                    op=OP.add)
                            t1r = t1v.rearrange("p (l f) -> p l f", f=64)
                            nc.vector.tensor_reduce(
                                out=beta[:, :L, 2:3], in_=t1r[:, :, 0:8],
                                axis=X, op=OP.add)
                            nc.vector.tensor_reduce(
                                out=beta[:, :L, 3:5],
                                in_=t1r[:, :, 8:56].rearrange(
                                    "p l (a f) -> p l a f", f=24),
                                axis=X, op=OP.add)
                            t5 = ework.tile([128, maxL], f32, tag="t5")
                            nc.vector.tensor_reduce(
                                out=t5[:, :L], in_=t1r[:, :, 56:64],
                                axis=X, op=OP.add)
                            t5b = t5[:, 0:L]
                            nc.vector.tensor_tensor(
                                out=beta[:, :L, 5:6], in0=beta[:, :L, 5:6],
                                in1=rv(t5b, [t5b.ap[0], [1, L], [0, 1]]),
                                op=OP.add)
                            wexp = ework.tile([128, maxL, 8], f32, tag="wexp")
                            nc.scalar.activation(
                                out=wexp[:, :L, :].rearrange(
                                    "p l f -> p (l f)"),
                                in_=bv.rearrange("p l f -> p (l f)"),
                                func=AF.Exp)
                            # U = [w (x) xj | w]
                            U = ework.tile([128, maxL, 72], f32, tag="U")
                            ub = U[:, 0:L, 0:64]
                            uo = rv(ub, [ub.ap[0], [72, L], [8, 8], [1, 8]])
                            xjr = rv(xj, [xj.ap[0], [64, L], [8, 8], [1, 8]])
                            wb = wexp[:, 0:L, :]
                            wv = rv(wb, [wb.ap[0], [8, L], [1, 8], [0, 8]])
                            nc.vector.tensor_tensor(out=uo, in0=xjr, in1=wv,
                                                    op=OP.mult)
                            nc.vector.tensor_copy(out=U[:, :L, 64:72],
                                                  in_=wb)
                            # S one-hot + per-tile matmuls
                            S = ework.tile([128, maxL, 128], f32, tag="S")
                            iov = rv(iota[:], [iota[:].ap[0], [0, L], [1, 128]])
                            nc.vector.tensor_tensor(out=S[:, :L, :], in0=iov,
                                                    in1=dstv, op=OP.is_equal)
                            if SUBSTAGE == "e":
                                done += L
                                continue
                            for l in range(L):
                                nc.tensor.matmul(
                                    out=acc[:], lhsT=S[:, l, :], rhs=U[:, l, :],
                                    start=first, stop=(done + l == tb_total - 1))
                                first = False
                            done += L
                        run_pos_in_de += tb_total
                        if SUBSTAGE == "full":
                            epilogue(b, acc, epip, ework)

            def softmax_div(acc, epip, ework):
                den = ework.tile([128, 8], f32, tag="den")
                nc.vector.tensor_scalar_add(den[:], acc[:, 64:72], EPS)
                rec = ework.tile([128, 8], f32, tag="rec")
                nc.vector.reciprocal(out=rec[:], in_=den[:])
                o64 = ework.tile([128, 64], f32, tag="o64")
                rb = rec[:]
                nc.vector.tensor_tensor(
                    out=o64[:].rearrange("p (a f) -> p a f", f=8),
                    in0=acc[:, 0:64].rearrange("p (a f) -> p a f", f=8),
                    in1=rv(rb, [rb.ap[0], [1, 8], [0, 8]]), op=OP.mult)
                pt = epip.tile([64, 128], f32, tag="pt")
                nc.tensor.transpose(out=pt[:], in_=o64[:], identity=ident[:])
                return pt

            def epi1(b, acc, epip, ework):
                pt = softmax_div(acc, epip, ework)
                cat = ework.tile([128, 128], f32, tag="cat")
                nc.vector.tensor_copy(out=cat[0:64, :], in_=pt[:])
                nc.sync.dma_start(out=cat[64:128, :],
                                  in_=cshardT[:, b * 128:(b + 1) * 128])
                ph = epip.tile([64, 128], f32, tag="ph")
                nc.tensor.matmul(out=ph[:], lhsT=wts["Wo1T"][:], rhs=cat[:],
                                 start=True, stop=True)
                hs = ework.tile([64, 128], f32, tag="hs")
                nc.scalar.activation(out=hs[:], in_=ph[:], func=AF.Relu,
                                     bias=wts["bo1"][:, 0:1])
                pc = epip.tile([64, 128], f32, tag="pc")
                nc.tensor.matmul(out=pc[:], lhsT=wts["Wo2T"][:], rhs=hs[:],
                                 start=True, stop=True)
                c2 = ework.tile([64, 128], f32, tag="c2")
                nc.scalar.activation(out=c2[:], in_=pc[:], func=AF.Identity,
                                     bias=wts["bo2"][:, 0:1])
                px = epip.tile([64, 128], f32, tag="pc")
                nc.tensor.matmul(out=px[:], lhsT=wts["Wl2T"][:], rhs=c2[:],
                                 start=True, stop=True)
                x2 = ework.tile([64, 128], f32, tag="x2")
                nc.scalar.activation(out=x2[:], in_=px[:], func=AF.Identity,
                                     bias=wts["bl2"][:, 0:1])
                pn = epip.tile([128, 64], f32, tag="pn")
                nc.tensor.transpose(out=pn[:], in_=x2[:],
                                    identity=ident[0:64, 0:64])
                xn = ework.tile([128, 64], f32, tag="xn2")
                nc.vector.tensor_copy(out=xn[:], in_=pn[:])
                nc.sync.dma_start(out=c2shard[b * 128:(b + 1) * 128, :],
                                  in_=xn[:])

            STAGE = os.environ.get("GAT_STAGE", "full")
            if STAGE in ("l1", "full"):
                gat_layer(runs1, NSUB_V, idx1, de1, vtable, NV_PAD,
                          bts["g1r"], bts["Q1r"], bts["PmQ1r"], CB_CORE, epi1)

            if STAGE in ("l1", "full") and SUBSTAGE == "full":
                nc.gpsimd.collective_compute(
                "AllGather", mybir.AluOpType.bypass,
                    replica_groups=[list(range(N_CORES))],
                    ins=[c2shard[:]], outs=[ctable[:]])

            score = const.tile([1, V_SHARD], f32)
            nc.vector.memset(score[:], 0.0)

            def epi2(b, acc, epip, ework):
                pt = softmax_div(acc, epip, ework)
                cat = ework.tile([128, 128], f32, tag="cat")
                nc.vector.tensor_copy(out=cat[0:64, :], in_=pt[:])
                nc.sync.dma_start(out=cat[64:128, :],
                                  in_=vshardT[:, b * 128:(b + 1) * 128])
                ph = epip.tile([64, 128], f32, tag="ph")
                nc.tensor.matmul(out=ph[:], lhsT=wts["Uo1T"][:], rhs=cat[:],
                                 start=True, stop=True)
                hs = ework.tile([64, 128], f32, tag="hs")
                nc.scalar.activation(out=hs[:], in_=ph[:], func=AF.Relu,
                                     bias=wts["ub1"][:, 0:1])
                pc = epip.tile([64, 128], f32, tag="pc")
                nc.tensor.matmul(out=pc[:], lhsT=wts["Uo2T"][:], rhs=hs[:],
                                 start=True, stop=True)
                v2 = ework.tile([64, 128], f32, tag="c2")
                nc.scalar.activation(out=v2[:], in_=pc[:], func=AF.Identity,
                                     bias=wts["ub2"][:, 0:1])
                ph3 = epip.tile([64, 128], f32, tag="ph")
                nc.tensor.matmul(out=ph3[:], lhsT=wts["hW1T"][:], rhs=v2[:],
                                 start=True, stop=True)
                h3 = ework.tile([64, 128], f32, tag="hs3")
                nc.scalar.activation(out=h3[:], in_=ph3[:], func=AF.Relu,
                                     bias=wts["hb1"][:, 0:1])
                psc = epip.tile([1, 128], f32, tag="psc")
                nc.tensor.matmul(out=psc[:], lhsT=wts["hW2"][:], rhs=h3[:],
                                 start=True, stop=True)
                nc.vector.tensor_copy(out=score[:, b * 128:(b + 1) * 128],
                                      in_=psc[:])

            if STAGE == "full":
                gat_layer(runs2, NSUB_C, idx2, de2, ctable, NC_PAD,
                          bts["g2r"], bts["Q2r"], bts["PmQ2r"], VB_CORE, epi2)

            nc.sync.dma_start(out=out_score[:], in_=score[:])

    nc.compile()

    in_maps = []
    for c in range(N_CORES):
        m = dict(host["weights"])
        m["vfT"] = host["vfT"][c]
        m["cfT"] = host["cfT"][c]
        idx1c, de1c = host["l1"][3][c]
        idx2c, de2c = host["l2"][3][c]
        for s in range(NSUB_V):
            m[f"idx1_{s}"] = idx1c[s]
        for s in range(NSUB_C):
            m[f"idx2_{s}"] = idx2c[s]
        m["de1"] = de1c
        m["de2"] = de2c
        in_maps.append(m)

    bench_iters = int(os.environ.get("GAT_BENCH", "0"))
    if bench_iters:
        results = _bench_via_pjrt(nc, in_maps, bench_iters)
    else:
        res = run_bass_kernel_spmd(nc, in_maps, core_ids=list(range(N_CORES)))
        results = res.results
    return np.concatenate(
        [results[c]["out_score"][0] for c in range(N_CORES)])


def _bench_via_pjrt(nc, in_maps, iters):
    """run_bass_via_pjrt clone that jits once, keeps inputs device-resident,
    and times steady-state executions (axon has no NTFF hook here)."""
    import time
    import jax
    import numpy as np
    from jax.sharding import Mesh, PartitionSpec
    from jax.experimental.shard_map import shard_map
    from concourse import bass2jax, mybir
    from concourse.bass2jax import (_bass_exec_p, install_neuronx_cc_hook,
                                    partition_id_tensor)

    install_neuronx_cc_hook()
    n_cores = len(in_maps)
    partition_name = (nc.partition_id_tensor.name
                      if nc.partition_id_tensor else None)
    in_names, out_names, out_avals, zero_outs = [], [], [], []
    for alloc in nc.m.functions[0].allocations:
        if not isinstance(alloc, mybir.MemoryLocationSet):
            continue
        name = alloc.memorylocations[0].name
        if alloc.kind == "ExternalInput":
            if name != partition_name:
                in_names.append(name)
        elif alloc.kind == "ExternalOutput":
            shape = tuple(alloc.tensor_shape)
            dtype = mybir.dt.np(alloc.dtype)
            out_names.append(name)
            out_avals.append(jax.core.ShapedArray(shape, dtype))
            zero_outs.append(np.zeros(shape, dtype))
    n_params = len(in_names)
    n_outs = len(out_avals)
    in_names.extend(out_names)
    if partition_name is not None:
        in_names.append(partition_name)
    donate = tuple(range(n_params, n_params + n_outs))

    def _body(*args):
        operands = list(args)
        if partition_name is not None:
            operands.append(partition_id_tensor())
        return tuple(_bass_exec_p.bind(
            *operands, out_avals=tuple(out_avals), in_names=tuple(in_names),
            out_names=tuple(out_names), lowering_input_output_aliases=(),
            sim_require_finite=True, sim_require_nnan=True, nc=nc))

    devices = jax.devices()[:n_cores]
    mesh = Mesh(np.asarray(devices), ("core",))
    sharded = jax.jit(
        shard_map(_body, mesh=mesh,
                  in_specs=(PartitionSpec("core"),) * (n_params + n_outs),
                  out_specs=(PartitionSpec("core"),) * n_outs,
                  check_rep=False),
        donate_argnums=donate, keep_unused=True)
    sharding = jax.sharding.NamedSharding(mesh, PartitionSpec("core"))
    concat_in = [
        jax.device_put(
            np.concatenate([np.asarray(in_maps[c][nm]) for c in range(n_cores)],
                           axis=0), sharding)
        for nm in in_names[:n_params]]
    concat_zero_np = [np.zeros((n_cores * z.shape[0], *z.shape[1:]), z.dtype)
                      for z in zero_outs]
    times = []
    out_arrs = None
    for it in range(iters):
        zs = [jax.device_put(z, sharding) for z in concat_zero_np]
        for z in zs:
            z.block_until_ready()
        t0 = time.perf_counter()
        out_arrs = sharded(*concat_in, *zs)
        for o in out_arrs:
            o.block_until_ready()
        times.append((time.perf_counter() - t0) * 1e9)
    global LAST_EXEC_NS, LAST_TIMES
    LAST_TIMES = times
    LAST_EXEC_NS = int(min(times))
    host_outs = [np.asarray(o) for o in out_arrs]
    return [
        {name: host_outs[i].reshape(n_cores, *out_avals[i].shape)[c]
         for i, name in enumerate(out_names)}
        for c in range(n_cores)]


def kernel(constraint_features, edge_indices, edge_features,
           variable_features, task_id, params):
    p = {k: (v if isinstance(v, dict) else np.asarray(v, np.float32))
         for k, v in params.items()}
    for k in ("v2c", "c2v"):
        p[k] = {kk: np.asarray(vv, np.float32) for kk, vv in p[k].items()}
    cf = np.asarray(constraint_features, np.float32)
    ef = np.asarray(edge_features, np.float32)
    vf = np.asarray(variable_features, np.float32)
    ei = np.asarray(edge_indices)
    tid = int(np.asarray(task_id))

    def fold(W1, b1, shift, scale):
        return ((W1 * scale[None, :]).astype(np.float32),
                (b1 + W1 @ (shift * scale)).astype(np.float32))

    cW1, cb1 = fold(p["cons_W1"], p["cons_b1"], p["cons_shift"],
                    p["cons_scale"])
    vW1, vb1 = fold(p["var_W1"], p["var_b1"], p["var_shift"], p["var_scale"])
    eW, eb = fold(p["edge_W"], p["edge_b"], p["edge_shift"], p["edge_scale"])
    assert np.abs(eb).max() == 0.0, "edge bias path not supported"
    ew = eW[:, 0]

    _, G_e1, g1 = _head_mats(p["v2c"]["att"])
    _, G_e2, g2 = _head_mats(p["c2v"]["att"])
    P1 = _lrelu(ew) @ G_e1
    Q1 = np.where(ew < 0, ew, 0.2 * ew) @ G_e1
    P2 = _lrelu(ew) @ G_e2
    Q2 = np.where(ew < 0, ew, 0.2 * ew) @ G_e2

    if tid == 1:
        hW1, hb1, hW2 = p["out1_W1"], p["out1_b1"], p["out1_W2"]
    else:
        hW1, hb1, hW2 = p["out2_W1"], p["out2_b1"], p["out2_W2"]

    weights = {
        "vW1T": vW1.T, "vb1": vb1.reshape(64, 1),
        "vW2T": p["var_W2"].T, "vb2": p["var_b2"].reshape(64, 1),
        "Wl1T": p["v2c"]["Wl"].T, "bl1": p["v2c"]["bl"].reshape(64, 1),
        "cW1T": cW1.T, "cb1": cb1.reshape(64, 1),
        "cW2T": p["cons_W2"].T, "cb2": p["cons_b2"].reshape(64, 1),
        "Wo1T": p["v2c"]["Wo1"].T, "bo1": p["v2c"]["bo1"].reshape(64, 1),
        "Wo2T": p["v2c"]["Wo2"].T, "bo2": p["v2c"]["bo2"].reshape(64, 1),
        "Wl2T": p["c2v"]["Wl"].T, "bl2": p["c2v"]["bl"].reshape(64, 1),
        "Uo1T": p["c2v"]["Wo1"].T, "ub1": p["c2v"]["bo1"].reshape(64, 1),
        "Uo2T": p["c2v"]["Wo2"].T, "ub2": p["c2v"]["bo2"].reshape(64, 1),
        "hW1T": hW1.T, "hb1": hb1.reshape(64, 1),
        "hW2": hW2.reshape(1, 64).T,
        "g1r": g1.reshape(1, 64), "g2r": g2.reshape(1, 64),
        "Q1r": Q1.reshape(1, 8), "PmQ1r": (P1 - Q1).reshape(1, 8),
        "Q2r": Q2.reshape(1, 8), "PmQ2r": (P2 - Q2).reshape(1, 8),
    }
    weights = {k: np.ascontiguousarray(v, dtype=np.float32)
               for k, v in weights.items()}

    cons_idx = ei[0].astype(np.int64)
    var_idx = ei[1].astype(np.int64)
    ef1 = ef[:, 0].astype(np.float32)

    n_edges = var_idx.shape[0]
    rot1 = np.empty((N_CORES, n_edges), np.int64)
    sub1 = np.empty((N_CORES, n_edges), np.int64)
    for c in range(N_CORES):
        vr = (var_idx - c * V_SHARD) % NV_PAD
        rot1[c] = vr % SUB
        sub1[c] = vr // SUB
    l1 = _prep_edges(cons_idx, rot1, sub1, ef1, CB_CORE, NSUB_V)

    loc2 = np.tile(cons_idx % SUB, (N_CORES, 1))
    sub2 = np.tile(cons_idx // SUB, (N_CORES, 1))
    l2 = _prep_edges(var_idx, loc2, sub2, ef1, VB_CORE, NSUB_C)

    vf_pad = np.zeros((NV_PAD, 15), np.float32)
    vf_pad[:N_VARS] = vf
    cf_pad = np.zeros((NC_PAD, 4), np.float32)
    cf_pad[:N_CONS] = cf
    vfT, cfT = [], []
    for c in range(N_CORES):
        ridx = (np.arange(NV_PAD) + c * V_SHARD) % NV_PAD
        vfT.append(np.ascontiguousarray(vf_pad[ridx].T))
        cfT.append(np.ascontiguousarray(
            cf_pad[c * C_SHARD:(c + 1) * C_SHARD].T))

    host = {"weights": weights, "vfT": vfT, "cfT": cfT, "l1": l1, "l2": l2}
    out = _build_and_run(host)
    return out[:N_VARS].astype(np.float32)
